# revision 15
# baseline (speedup 1.0000x reference)
"""Multi-head attention kernel for 8 TRN2 NeuronCores.

Problem: b=2, n=2048, d=1024, heads=16, hd=64.
  q/k/v = x @ W{q,k,v}.T (+ zero bias)
  per head: softmax(q k^T / sqrt(d)) @ v
  out = concat @ Wo.T (+ zero bias)

Sharding (8 cores): data-parallel over batch (2) x tensor-parallel over
heads (16 heads -> 4 groups of 4). Core c handles batch c//4, heads
4*(c%4) .. 4*(c%4)+3 (feature slice of 256 columns). Wo is applied
row-parallel: each core emits a partial output (transposed [d, n],
bf16); the host sums the 4 partials per batch and untransposes.

v3 design (measured HW evolution from the f32r baseline at ~267us and
the v2 rewrite at ~270us):
 - Everything bf16 (same PE rate as f32r, half the DMA + SBUF). Host
   pre-transposes/casts: xT (d,n), w{q,k,v}T (d,256), woT (256,d).
 - Steady state is ACT-paced: per round (head-pair, 512-q block, one
   128-key chunk) the PE does 2 scores matmuls (row-tiled K=64 pair,
   ~390ns) + 2 AV matmuls (~430ns) against one FD=1024 exp (~1114ns).
   The ~365ns/round PE slack is filled by "work items" (projection
   half-blocks, V tiles, Wo chunks) emitted BETWEEN the scores and AV
   matmuls of each round - engine queues are FIFO, so emission order
   controls exactly what the PE does during the exp shadow.
 - Minimal lead-in: only K^T fc0 block0 (streamed behind the xT DMA)
   + Q^T fc0 qb0 + V(0..1) gate the first exp; all other projections
   ride inside passes as work items.
 - Wo for q-block qb is emitted as per-do items inside the next
   q-block's first pass (round >= 4, giving the normalize chain time);
   the last q-block's Wo runs at the tail with its PSUM->SBUF casts on
   ScalarE (idle there) instead of DVE.
 - softmax denominators via the ones-column of V_aug (row hd of avo);
   normalize: copy avo out of PSUM fast, reciprocal in a [128, 4]
   partition-scattered layout via a small SBUF DMA round-trip,
   partition_broadcast on GpSimd, multiply on DVE -> outT bf16.
 - PSUM: scps 2 bufs x [128,2,512]f32 (2 banks each) + avo pair
   (2 banks) + pjps 2 bufs x 1 bank for projection/Wo items = 8 banks.

Biases are structurally zero in this problem spec and are skipped.
"""

import numpy as np

HEADS = 16
D = 1024
N = 2048
B = 2
N_CORES = 8
HPC = HEADS // (N_CORES // B)  # heads per core = 4
HD = D // HEADS                # 64
F = HPC * HD                   # 256 features per core
P = 128


def build_nc(n=N, d=D, hpc=HPC, hd=HD):
    """Build the per-core Bass program (SPMD: same program on all 8 cores)."""
    import concourse.bass as bass
    import concourse.tile as tile
    from concourse import bacc, mybir

    f32 = mybir.dt.float32
    bf16 = mybir.dt.bfloat16
    f = hpc * hd            # per-core feature count (256)
    FC = f // P             # feature chunks / head pairs (2)
    DC = d // P             # contraction chunks over d (8)
    NT = n // P             # key chunks (16)
    QB = 512                # q-block width
    NQB = n // QB           # 4
    scale = 1.0 / float(np.sqrt(np.float32(d)))

    nc = bacc.Bacc("TRN2")

    xT = nc.declare_dram_parameter("xT", [d, n], bf16, isOutput=False)
    wqT = nc.declare_dram_parameter("wqT", [d, f], bf16, isOutput=False)
    wkT = nc.declare_dram_parameter("wkT", [d, f], bf16, isOutput=False)
    wvT = nc.declare_dram_parameter("wvT", [d, f], bf16, isOutput=False)
    woT = nc.declare_dram_parameter("woT", [f, d], bf16, isOutput=False)
    out = nc.declare_dram_parameter("out", [d, n], bf16, isOutput=True)

    xT_c = xT.rearrange("(c p) n -> c p n", p=P)
    wqT_c = wqT.rearrange("(c p) f -> c p f", p=P)
    wkT_c = wkT.rearrange("(c p) f -> c p f", p=P)
    wvT_c = wvT.rearrange("(c p) f -> c p f", p=P)
    woT_c = woT.rearrange("(c p) n -> c p n", p=P)

    with tile.TileContext(nc) as tc:
        with (
            tc.tile_pool(name="qkv", bufs=1) as qkv,
            tc.tile_pool(name="outT", bufs=1) as outp,
            tc.tile_pool(name="pt", bufs=2) as ptp,
            tc.tile_pool(name="norm", bufs=1) as normp,
            tc.tile_pool(name="xw", bufs=1) as xw,
            tc.tile_pool(name="wosb", bufs=4) as wosbp,
            tc.tile_pool(name="scps", bufs=2, space="PSUM") as scps,
            tc.tile_pool(name="avps", bufs=1, space="PSUM") as avps,
            tc.tile_pool(name="pjps", bufs=2, space="PSUM") as pjps,
        ):
            QT_sb = qkv.tile([P, FC, n], bf16)
            KT_sb = qkv.tile([P, FC, n], bf16)
            V_sb = qkv.tile([P, NT, hpc, hd + 1], bf16)
            outT_sb = outp.tile([P, FC, n], bf16)
            woT_sb = outp.tile([P, FC, d], bf16)
            # ones column of V_aug (accumulates softmax denominators in AV)
            ones_c = outp.tile([P, 1], bf16)
            nc.vector.memset(ones_c[:], 1.0)
            nc.vector.tensor_copy(
                V_sb[:, :, :, hd : hd + 1],
                ones_c.to_broadcast([P, NT, hpc, 1]),
            )

            xT_r = xw.tile([P, DC, n], bf16)
            wqT_r = xw.tile([P, DC, f], bf16)
            wkT_r = xw.tile([P, DC, f], bf16)
            wvT_r = xw.tile([P, DC, f], bf16)

            # DMA order: wk chunk 0 (feeds warm-up + KT b0), then the whole
            # xT stream back-to-back (it paces the lead-in), then the rest.
            nc.sync.dma_start(out=wkT_r[:, 0, :], in_=wkT_c[0])
            for dc in range(DC):
                nc.sync.dma_start(out=xT_r[:, dc, :], in_=xT_c[dc])
            for dc in range(1, DC):
                nc.sync.dma_start(out=wkT_r[:, dc, :], in_=wkT_c[dc])
            for dc in range(DC):
                nc.sync.dma_start(out=wqT_r[:, dc, :], in_=wqT_c[dc])
            for dc in range(DC):
                nc.sync.dma_start(out=wvT_r[:, dc, :], in_=wvT_c[dc])
            for fc in range(FC):
                nc.sync.dma_start(out=woT_sb[:, fc, :], in_=woT_c[fc])

            def warm_up(k):
                # throwaway matmuls on the first wk chunk: keep the PE_HAM
                # activity window busy so real work runs at 2.4 GHz instead
                # of the cold 1.2 GHz default.
                for w in range(k):
                    warm = pjps.tile([P, f], f32, tag="pj", name="warm")
                    nc.tensor.matmul(
                        warm[:],
                        wkT_r[:, 0, 0:P],
                        wkT_r[:, 0, :],
                        start=True,
                        stop=True,
                    )

            # ---- work items ----
            # Lead-in projections are split into two 4-dc halves: h0 (needs
            # xT chunks 0-3 only) accumulates in PSUM, is copied to an SBUF
            # f32 stage and releases its PSUM buffer immediately - so the
            # DMA-paced phase can chew through many h0 items on the 2-buf
            # pjps pool; h1 (chunks 4-7) re-accumulates and DVE-adds the
            # stage into the bf16 destination. Steady-state QT blocks use
            # 2-dc quarter items on one held PSUM tile instead.
            stg = xw.tile([P, 10, QB], f32)     # KT b0-3 x2fc, QT b0 x2fc

            vstg = xw.tile([P, NT, f], f32)
            quarter_state = {}

            def qk_h0(w_sb, fc, qc, si):
                ps = pjps.tile([P, QB], f32, tag="pj")
                sl = slice(qc * QB, (qc + 1) * QB)
                for dc in range(4):
                    nc.tensor.matmul(
                        ps[:],
                        w_sb[:, dc, fc * P : (fc + 1) * P],
                        xT_r[:, dc, sl],
                        start=(dc == 0),
                        stop=(dc == 3),
                    )
                nc.vector.tensor_copy(stg[:, si, :], ps[:])

            def qk_h1(w_sb, dest, fc, qc, si):
                ps = pjps.tile([P, QB], f32, tag="pj")
                sl = slice(qc * QB, (qc + 1) * QB)
                for dc in range(4, DC):
                    nc.tensor.matmul(
                        ps[:],
                        w_sb[:, dc, fc * P : (fc + 1) * P],
                        xT_r[:, dc, sl],
                        start=(dc == 4),
                        stop=(dc == DC - 1),
                    )
                nc.vector.tensor_add(dest[:, fc, sl], stg[:, si, :], ps[:])

            def v_h0(nt):
                ps = pjps.tile([P, QB], f32, tag="pj")
                for dc in range(4):
                    nc.tensor.matmul(
                        ps[:, 0:f],
                        xT_r[:, dc, nt * P : (nt + 1) * P],
                        wvT_r[:, dc, :],
                        start=(dc == 0),
                        stop=(dc == 3),
                    )
                nc.vector.tensor_copy(vstg[:, nt, :], ps[:, 0:f])

            def v_h1(nt):
                ps = pjps.tile([P, QB], f32, tag="pj")
                for dc in range(4, DC):
                    nc.tensor.matmul(
                        ps[:, 0:f],
                        xT_r[:, dc, nt * P : (nt + 1) * P],
                        wvT_r[:, dc, :],
                        start=(dc == 4),
                        stop=(dc == DC - 1),
                    )
                nc.vector.tensor_add(
                    V_sb[:, nt, :, 0:hd],
                    vstg[:, nt, :].rearrange("p (h e) -> p h e", h=hpc),
                    ps[:, 0:f].rearrange("p (h e) -> p h e", h=hpc),
                )

            def qk_q(w_sb, dest, fc, qc, quarter):
                """2-dc quarter of a steady-state projection block; the
                PSUM tile is held across the 4 quarters."""
                key = (id(w_sb), fc, qc)
                sl = slice(qc * QB, (qc + 1) * QB)
                if quarter == 0:
                    ps = pjps.tile([P, QB], f32, tag="pj")
                    quarter_state[key] = ps
                else:
                    ps = quarter_state[key]
                for dc in range(2 * quarter, 2 * quarter + 2):
                    nc.tensor.matmul(
                        ps[:],
                        w_sb[:, dc, fc * P : (fc + 1) * P],
                        xT_r[:, dc, sl],
                        start=(dc == 0),
                        stop=(dc == DC - 1),
                    )
                if quarter == 3:
                    del quarter_state[key]
                    nc.vector.tensor_copy(dest[:, fc, sl], ps[:])

            def wo_item(qb, do, tail=False):
                """One do-chunk of the output projection for q-block qb
                (woT stationary; emits the partial TRANSPOSED [128, QB])."""
                q0 = qb * QB
                ps = pjps.tile([P, QB], f32, tag="pj")
                for fc in range(FC):
                    nc.tensor.matmul(
                        ps[:],
                        woT_sb[:, fc, do * P : (do + 1) * P],
                        outT_sb[:, fc, q0 : q0 + QB],
                        start=(fc == 0),
                        stop=(fc == FC - 1),
                    )
                ob = wosbp.tile([P, QB], bf16, tag="ob")
                if tail:
                    nc.scalar.copy(ob[:], ps[:])
                else:
                    nc.vector.tensor_copy(ob[:], ps[:])
                nc.sync.dma_start(
                    out=out[do * P : (do + 1) * P, q0 : q0 + QB],
                    in_=ob[:],
                )

            def pass_end(fc, qb, avos):
                """Free avo fast, then normalize rows 0..hd-1 by row hd (the
                softmax sums). reciprocal is single-lane-slow on a [1, QB]
                row, so scatter the sums across partitions via a small SBUF
                DMA round-trip first."""
                q0 = qb * QB
                for hi in range(2):
                    po = hi * hd
                    av_sb = normp.tile([hd + 1, QB], f32, tag=f"av_sb{hi}")
                    nc.vector.tensor_copy(av_sb[:], avos[hi][:])
                    rsh = normp.tile([P, QB // P], f32, tag=f"rsh{hi}")
                    nc.sync.dma_start(out=rsh[:], in_=av_sb[hd : hd + 1, :])
                    rsh2 = normp.tile([P, QB // P], f32, tag=f"rsh2{hi}")
                    nc.vector.reciprocal(rsh2[:], rsh[:])
                    recip = normp.tile([1, QB], f32, tag=f"recip{hi}")
                    nc.sync.dma_start(out=recip[:], in_=rsh2[:])
                    bc = normp.tile([hd, QB], f32, tag=f"bc{hi}")
                    nc.gpsimd.partition_broadcast(bc[:], recip[:])
                    nc.vector.tensor_mul(
                        outT_sb[po : po + hd, fc, q0 : q0 + QB],
                        av_sb[0:hd, :],
                        bc[:],
                    )

            def do_pass(fc, qb, fillers):
                """Attention pass for head pair fc (heads 2fc, 2fc+1) on
                q-block qb. fillers[r] = list of work items emitted between
                round r's scores and AV matmuls (they run in the exp
                shadow; the engine queue is FIFO so placement = pacing)."""
                q0 = qb * QB
                avos = [
                    avps.tile([hd + 1, QB], f32, tag=f"avo{i}", name=f"avo{i}")
                    for i in range(2)
                ]
                for kc in range(NT):
                    sc = scps.tile([P, 2, QB], f32, tag="sc")
                    for hi in range(2):
                        p0 = hi * hd
                        nc.tensor.matmul(
                            sc[:, hi, :],
                            KT_sb[p0 : p0 + hd, fc, kc * P : (kc + 1) * P],
                            QT_sb[p0 : p0 + hd, fc, q0 : q0 + QB],
                            start=True,
                            stop=True,
                        )
                    pt = ptp.tile([P, 2, QB], bf16, tag="pt")
                    nc.scalar.activation(
                        pt[:], sc[:], mybir.ActivationFunctionType.Exp,
                        scale=scale,
                    )
                    for item in fillers.get(kc, []):
                        item()
                    for hi in range(2):
                        nc.tensor.matmul(
                            avos[hi][:],
                            V_sb[:, kc, 2 * fc + hi, :],
                            pt[:, hi, :],
                            start=(kc == 0),
                            stop=(kc == NT - 1),
                        )
                pass_end(fc, qb, avos)

            # ---- emission schedule (emission order = scheduler priority;
            # actual execution order is dependency-driven, so low-priority
            # h0 items automatically fill the xT-DMA wait gaps) ----
            def KH1(fc, qc, si):
                return lambda: qk_h1(wkT_r, KT_sb, fc, qc, si)

            def QH1(fc, qc, si):
                return lambda: qk_h1(wqT_r, QT_sb, fc, qc, si)

            def QQ(fc, qc, quarter):
                return lambda: qk_q(wqT_r, QT_sb, fc, qc, quarter)

            def VH1(nt):
                return lambda: v_h1(nt)

            def WO(qb, do, tail=False):
                return lambda: wo_item(qb, do, tail)

            # critical chain to the first exp: KT b0 + QT b0 (fc0), whose
            # h1 halves run the moment xT chunk 7 lands.
            warm_up(16)
            qk_h0(wkT_r, 0, 0, 0)
            qk_h1(wkT_r, KT_sb, 0, 0, 0)
            qk_h0(wqT_r, 0, 0, 8)
            qk_h1(wqT_r, QT_sb, 0, 0, 8)
            v_h0(0)
            v_h0(1)
            # bulk h0 halves: only need chunks 0-3 + their weights; they
            # fill the rest of the DMA phase and early-round slack.
            qk_h0(wkT_r, 0, 1, 1)
            qk_h0(wkT_r, 0, 2, 2)
            qk_h0(wkT_r, 0, 3, 3)
            qk_h0(wkT_r, 1, 0, 4)
            qk_h0(wkT_r, 1, 1, 5)
            qk_h0(wkT_r, 1, 2, 6)
            qk_h0(wkT_r, 1, 3, 7)
            qk_h0(wqT_r, 1, 0, 9)
            for nt in range(2, NT):
                v_h0(nt)

            # pass(0,0): V h1 paced 2 chunks ahead of its kc; KT fc0 h1
            # blocks land before their first use (round 4j); KT/QT fc1 b0
            # h1 land before pass(1,0) round 0.
            f00 = {0: [VH1(0), VH1(1), VH1(2)]}
            for kc in range(1, 14):
                f00[kc] = [VH1(kc + 2)]
            f00[1].append(KH1(0, 1, 1))
            f00[5].append(KH1(0, 2, 2))
            f00[9].append(KH1(0, 3, 3))
            f00[11].append(KH1(1, 0, 4))
            f00[13].append(QH1(1, 0, 9))
            do_pass(0, 0, f00)

            # pass(1,0): KT fc1 h1 blocks before their round-4j use; QT
            # qb1 quarter items for both pairs (two blocks straddle the
            # pjps pool on interleaved rounds - exactly 2 held tiles).
            f10 = {
                0: [KH1(1, 1, 5)],
                2: [KH1(1, 2, 6)],
                4: [KH1(1, 3, 7)],
            }
            for i in range(4):
                f10[5 + 2 * i] = [QQ(1, 1, i)]
                f10[6 + 2 * i] = [QQ(0, 1, i)]
            do_pass(1, 0, f10)

            # q-blocks 1..3: wo(qb-1) split 4/4 across the two passes
            # (first use 8 rounds after the normalize chain starts); the
            # next q-block's QT blocks ride as quarter items.
            for qb in range(1, NQB):
                fa = {}
                if qb < NQB - 1:
                    for i in range(4):
                        fa[2 * i] = [QQ(1, qb + 1, i)]
                for i in range(4):
                    fa[8 + 2 * i] = [WO(qb - 1, i)]
                do_pass(0, qb, fa)
                fb = {}
                for i in range(3):
                    fb[2 * i] = [WO(qb - 1, 4 + i)]
                fb[14] = [WO(qb - 1, 7)]
                if qb < NQB - 1:
                    for i in range(4):
                        fb[7 + 2 * i] = [QQ(0, qb + 1, i)]
                do_pass(1, qb, fb)
            # tail: keep the PE warm through the last normalize chain,
            # then the last q-block's Wo with casts on the idle ScalarE.
            warm_up(8)
            for do in range(d // P):
                wo_item(NQB - 1, do, tail=True)
    nc.finalize()
    return nc


def make_in_maps(x, Wq, Wk, Wv, Wo):
    """Shard full inputs into per-core DRAM parameter maps (bf16)."""
    import ml_dtypes

    bf = ml_dtypes.bfloat16
    x = np.asarray(x, dtype=np.float32)
    Wq = np.asarray(Wq, dtype=np.float32)
    Wk = np.asarray(Wk, dtype=np.float32)
    Wv = np.asarray(Wv, dtype=np.float32)
    Wo = np.asarray(Wo, dtype=np.float32)
    xTs = [np.ascontiguousarray(x[b].T).astype(bf) for b in range(B)]
    WqT, WkT, WvT = Wq.T, Wk.T, Wv.T
    in_maps = []
    for c in range(N_CORES):
        b, g = c // (N_CORES // B), c % (N_CORES // B)
        fs = slice(g * F, (g + 1) * F)
        in_maps.append(
            {
                "xT": xTs[b],
                "wqT": np.ascontiguousarray(WqT[:, fs]).astype(bf),
                "wkT": np.ascontiguousarray(WkT[:, fs]).astype(bf),
                "wvT": np.ascontiguousarray(WvT[:, fs]).astype(bf),
                "woT": np.ascontiguousarray(Wo[:, fs].T).astype(bf),
            }
        )
    return in_maps


_NC_CACHE = {}


def run(x, Wq, Wk, Wv, Wo, trace=False, **kw):
    from concourse.bass_utils import run_bass_kernel_spmd

    if "nc" not in _NC_CACHE:
        _NC_CACHE["nc"] = build_nc()
    nc = _NC_CACHE["nc"]
    in_maps = make_in_maps(x, Wq, Wk, Wv, Wo)
    res = run_bass_kernel_spmd(
        nc, in_maps, core_ids=list(range(N_CORES)), trace=trace, **kw
    )
    parts = [
        np.asarray(res.results[i]["out"]).astype(np.float32)
        for i in range(N_CORES)
    ]
    gpb = N_CORES // B
    # per-core partials are transposed [d, n]: sum the group, then untranspose
    full = np.stack(
        [
            sum(parts[b * gpb + 1 : (b + 1) * gpb], parts[b * gpb]).T
            for b in range(B)
        ]
    )
    return np.ascontiguousarray(full, dtype=np.float32), res


def kernel(x, Wq, bq, Wk, bk, Wv, bv, Wo, bo):
    full, _ = run(x, Wq, Wk, Wv, Wo)
    return full


# revision 16
# speedup vs baseline: 1.1755x; 1.1755x over previous
"""Multi-head attention kernel for 8 TRN2 NeuronCores.

Problem: b=2, n=2048, d=1024, heads=16, hd=64.
  q/k/v = x @ W{q,k,v}.T (+ zero bias)
  per head: softmax(q k^T / sqrt(d)) @ v
  out = concat @ Wo.T (+ zero bias)

Sharding (8 cores): data-parallel over batch (2) x tensor-parallel over
heads (16 heads -> 4 groups of 4). Core c handles batch c//4, heads
4*(c%4) .. 4*(c%4)+3 (feature slice of 256 columns). Wo is applied
row-parallel: each core emits a partial output (transposed [d, n],
bf16); the host sums the 4 partials per batch and untransposes.

v3 design (measured HW evolution from the f32r baseline at ~267us and
the v2 rewrite at ~270us):
 - Everything bf16 (same PE rate as f32r, half the DMA + SBUF). Host
   pre-transposes/casts: xT (d,n), w{q,k,v}T (d,256), woT (256,d).
 - Steady state is ACT-paced: per round (head-pair, 512-q block, one
   128-key chunk) the PE does 2 scores matmuls (row-tiled K=64 pair,
   ~390ns) + 2 AV matmuls (~430ns) against one FD=1024 exp (~1114ns).
   The ~365ns/round PE slack is filled by "work items" (projection
   half-blocks, V tiles, Wo chunks) emitted BETWEEN the scores and AV
   matmuls of each round - engine queues are FIFO, so emission order
   controls exactly what the PE does during the exp shadow.
 - Minimal lead-in: only K^T fc0 block0 (streamed behind the xT DMA)
   + Q^T fc0 qb0 + V(0..1) gate the first exp; all other projections
   ride inside passes as work items.
 - Wo for q-block qb is emitted as per-do items inside the next
   q-block's first pass (round >= 4, giving the normalize chain time);
   the last q-block's Wo runs at the tail with its PSUM->SBUF casts on
   ScalarE (idle there) instead of DVE.
 - softmax denominators via the ones-column of V_aug (row hd of avo);
   normalize: copy avo out of PSUM fast, reciprocal in a [128, 4]
   partition-scattered layout via a small SBUF DMA round-trip,
   partition_broadcast on GpSimd, multiply on DVE -> outT bf16.
 - PSUM: scps 2 bufs x [128,2,512]f32 (2 banks each) + avo pair
   (2 banks) + pjps 2 bufs x 1 bank for projection/Wo items = 8 banks.

Biases are structurally zero in this problem spec and are skipped.
"""

import numpy as np

HEADS = 16
D = 1024
N = 2048
B = 2
N_CORES = 8
HPC = HEADS // (N_CORES // B)  # heads per core = 4
HD = D // HEADS                # 64
F = HPC * HD                   # 256 features per core
P = 128


def build_nc(n=N, d=D, hpc=HPC, hd=HD):
    """Build the per-core Bass program (SPMD: same program on all 8 cores)."""
    import concourse.bass as bass
    import concourse.tile as tile
    from concourse import bacc, mybir

    f32 = mybir.dt.float32
    bf16 = mybir.dt.bfloat16
    f = hpc * hd            # per-core feature count (256)
    FC = f // P             # feature chunks / head pairs (2)
    DC = d // P             # contraction chunks over d (8)
    NT = n // P             # key chunks (16)
    QB = 512                # q-block width
    NQB = n // QB           # 4
    scale = 1.0 / float(np.sqrt(np.float32(d)))

    nc = bacc.Bacc("TRN2")

    xT = nc.declare_dram_parameter("xT", [d, n], bf16, isOutput=False)
    wqT = nc.declare_dram_parameter("wqT", [d, f], bf16, isOutput=False)
    wkT = nc.declare_dram_parameter("wkT", [d, f], bf16, isOutput=False)
    wvT = nc.declare_dram_parameter("wvT", [d, f], bf16, isOutput=False)
    woT = nc.declare_dram_parameter("woT", [f, d], bf16, isOutput=False)
    out = nc.declare_dram_parameter("out", [d, n], bf16, isOutput=True)

    xT_c = xT.rearrange("(c p) n -> c p n", p=P)
    wqT_c = wqT.rearrange("(c p) f -> c p f", p=P)
    wkT_c = wkT.rearrange("(c p) f -> c p f", p=P)
    wvT_c = wvT.rearrange("(c p) f -> c p f", p=P)
    woT_c = woT.rearrange("(c p) n -> c p n", p=P)

    with tile.TileContext(nc) as tc:
        with (
            tc.tile_pool(name="qkv", bufs=1) as qkv,
            tc.tile_pool(name="outT", bufs=1) as outp,
            tc.tile_pool(name="pt", bufs=2) as ptp,
            tc.tile_pool(name="norm", bufs=1) as normp,
            tc.tile_pool(name="xw", bufs=1) as xw,
            tc.tile_pool(name="wosb", bufs=4) as wosbp,
            tc.tile_pool(name="scps", bufs=2, space="PSUM") as scps,
            tc.tile_pool(name="avps", bufs=1, space="PSUM") as avps,
            tc.tile_pool(name="pjps", bufs=2, space="PSUM") as pjps,
        ):
            QT_sb = qkv.tile([P, FC, n], bf16)
            KT_sb = qkv.tile([P, FC, n], bf16)
            V_sb = qkv.tile([P, NT, hpc, hd + 1], bf16)
            outT_sb = outp.tile([P, FC, n], bf16)
            woT_sb = outp.tile([P, FC, d], bf16)
            # ones column of V_aug (accumulates softmax denominators in AV)
            ones_c = outp.tile([P, 1], bf16)
            nc.vector.memset(ones_c[:], 1.0)
            nc.vector.tensor_copy(
                V_sb[:, :, :, hd : hd + 1],
                ones_c.to_broadcast([P, NT, hpc, 1]),
            )

            xT_r = xw.tile([P, DC, n], bf16)
            wqT_r = xw.tile([P, DC, f], bf16)
            wkT_r = xw.tile([P, DC, f], bf16)
            wvT_r = xw.tile([P, DC, f], bf16)

            # wk + xT interleaved per chunk (measured faster than issuing
            # the xT stream back-to-back; the queue pipelines them).
            for dc in range(DC):
                nc.sync.dma_start(out=wkT_r[:, dc, :], in_=wkT_c[dc])
                nc.sync.dma_start(out=xT_r[:, dc, :], in_=xT_c[dc])
            for dc in range(DC):
                nc.sync.dma_start(out=wqT_r[:, dc, :], in_=wqT_c[dc])
            for dc in range(DC):
                nc.sync.dma_start(out=wvT_r[:, dc, :], in_=wvT_c[dc])
            for fc in range(FC):
                nc.sync.dma_start(out=woT_sb[:, fc, :], in_=woT_c[fc])

            def warm_up(k):
                # throwaway matmuls on the first wk chunk: keep the PE_HAM
                # activity window busy so real work runs at 2.4 GHz instead
                # of the cold 1.2 GHz default.
                for w in range(k):
                    warm = pjps.tile([P, f], f32, tag="pj", name="warm")
                    nc.tensor.matmul(
                        warm[:],
                        wkT_r[:, 0, 0:P],
                        wkT_r[:, 0, :],
                        start=True,
                        stop=True,
                    )

            # ---- work items ----
            # Lead-in projections are split into two 4-dc halves: h0 (needs
            # xT chunks 0-3 only) accumulates in PSUM, is copied to an SBUF
            # f32 stage and releases its PSUM buffer immediately - so the
            # DMA-paced phase can chew through many h0 items on the 2-buf
            # pjps pool; h1 (chunks 4-7) re-accumulates and DVE-adds the
            # stage into the bf16 destination. Steady-state QT blocks use
            # 2-dc quarter items on one held PSUM tile instead.
            stg = xw.tile([P, 10, QB], f32)     # KT b0-3 x2fc, QT b0 x2fc

            vstg = xw.tile([P, NT, f], f32)
            quarter_state = {}

            def qk_h0(w_sb, fc, qc, si):
                ps = pjps.tile([P, QB], f32, tag="pj")
                sl = slice(qc * QB, (qc + 1) * QB)
                for dc in range(4):
                    nc.tensor.matmul(
                        ps[:],
                        w_sb[:, dc, fc * P : (fc + 1) * P],
                        xT_r[:, dc, sl],
                        start=(dc == 0),
                        stop=(dc == 3),
                    )
                nc.vector.tensor_copy(stg[:, si, :], ps[:])

            def qk_h1(w_sb, dest, fc, qc, si):
                ps = pjps.tile([P, QB], f32, tag="pj")
                sl = slice(qc * QB, (qc + 1) * QB)
                for dc in range(4, DC):
                    nc.tensor.matmul(
                        ps[:],
                        w_sb[:, dc, fc * P : (fc + 1) * P],
                        xT_r[:, dc, sl],
                        start=(dc == 4),
                        stop=(dc == DC - 1),
                    )
                nc.vector.tensor_add(dest[:, fc, sl], stg[:, si, :], ps[:])

            def v_h0(nt):
                ps = pjps.tile([P, QB], f32, tag="pj")
                for dc in range(4):
                    nc.tensor.matmul(
                        ps[:, 0:f],
                        xT_r[:, dc, nt * P : (nt + 1) * P],
                        wvT_r[:, dc, :],
                        start=(dc == 0),
                        stop=(dc == 3),
                    )
                nc.vector.tensor_copy(vstg[:, nt, :], ps[:, 0:f])

            def v_h1(nt):
                ps = pjps.tile([P, QB], f32, tag="pj")
                for dc in range(4, DC):
                    nc.tensor.matmul(
                        ps[:, 0:f],
                        xT_r[:, dc, nt * P : (nt + 1) * P],
                        wvT_r[:, dc, :],
                        start=(dc == 4),
                        stop=(dc == DC - 1),
                    )
                nc.vector.tensor_add(
                    V_sb[:, nt, :, 0:hd],
                    vstg[:, nt, :].rearrange("p (h e) -> p h e", h=hpc),
                    ps[:, 0:f].rearrange("p (h e) -> p h e", h=hpc),
                )

            def qk_q(w_sb, dest, fc, qc, quarter):
                """2-dc quarter of a steady-state projection block; the
                PSUM tile is held across the 4 quarters."""
                key = (id(w_sb), fc, qc)
                sl = slice(qc * QB, (qc + 1) * QB)
                if quarter == 0:
                    ps = pjps.tile([P, QB], f32, tag="pj")
                    quarter_state[key] = ps
                else:
                    ps = quarter_state[key]
                for dc in range(2 * quarter, 2 * quarter + 2):
                    nc.tensor.matmul(
                        ps[:],
                        w_sb[:, dc, fc * P : (fc + 1) * P],
                        xT_r[:, dc, sl],
                        start=(dc == 0),
                        stop=(dc == DC - 1),
                    )
                if quarter == 3:
                    del quarter_state[key]
                    nc.vector.tensor_copy(dest[:, fc, sl], ps[:])

            def wo_item(qb, do, tail=False):
                """One do-chunk of the output projection for q-block qb
                (woT stationary; emits the partial TRANSPOSED [128, QB])."""
                q0 = qb * QB
                ps = pjps.tile([P, QB], f32, tag="pj")
                for fc in range(FC):
                    nc.tensor.matmul(
                        ps[:],
                        woT_sb[:, fc, do * P : (do + 1) * P],
                        outT_sb[:, fc, q0 : q0 + QB],
                        start=(fc == 0),
                        stop=(fc == FC - 1),
                    )
                ob = wosbp.tile([P, QB], bf16, tag="ob")
                if tail:
                    nc.scalar.copy(ob[:], ps[:])
                else:
                    nc.vector.tensor_copy(ob[:], ps[:])
                nc.sync.dma_start(
                    out=out[do * P : (do + 1) * P, q0 : q0 + QB],
                    in_=ob[:],
                )

            def pass_end(fc, qb, avos):
                """Free avo fast, then normalize rows 0..hd-1 by row hd (the
                softmax sums). reciprocal is single-lane-slow on a [1, QB]
                row, so scatter the sums across partitions via a small SBUF
                DMA round-trip first."""
                q0 = qb * QB
                for hi in range(2):
                    po = hi * hd
                    av_sb = normp.tile([hd + 1, QB], f32, tag=f"av_sb{hi}")
                    nc.vector.tensor_copy(av_sb[:], avos[hi][:])
                    rsh = normp.tile([P, QB // P], f32, tag=f"rsh{hi}")
                    nc.sync.dma_start(out=rsh[:], in_=av_sb[hd : hd + 1, :])
                    rsh2 = normp.tile([P, QB // P], f32, tag=f"rsh2{hi}")
                    nc.vector.reciprocal(rsh2[:], rsh[:])
                    recip = normp.tile([1, QB], f32, tag=f"recip{hi}")
                    nc.sync.dma_start(out=recip[:], in_=rsh2[:])
                    bc = normp.tile([hd, QB], f32, tag=f"bc{hi}")
                    nc.gpsimd.partition_broadcast(bc[:], recip[:])
                    nc.vector.tensor_mul(
                        outT_sb[po : po + hd, fc, q0 : q0 + QB],
                        av_sb[0:hd, :],
                        bc[:],
                    )

            def do_pass(fc, qb, fillers):
                """Attention pass for head pair fc (heads 2fc, 2fc+1) on
                q-block qb. fillers[r] = list of work items emitted between
                round r's scores and AV matmuls (they run in the exp
                shadow; the engine queue is FIFO so placement = pacing)."""
                q0 = qb * QB
                avos = [
                    avps.tile([hd + 1, QB], f32, tag=f"avo{i}", name=f"avo{i}")
                    for i in range(2)
                ]
                for kc in range(NT):
                    sc = scps.tile([P, 2, QB], f32, tag="sc")
                    for hi in range(2):
                        p0 = hi * hd
                        nc.tensor.matmul(
                            sc[:, hi, :],
                            KT_sb[p0 : p0 + hd, fc, kc * P : (kc + 1) * P],
                            QT_sb[p0 : p0 + hd, fc, q0 : q0 + QB],
                            start=True,
                            stop=True,
                        )
                    pt = ptp.tile([P, 2, QB], bf16, tag="pt")
                    nc.scalar.activation(
                        pt[:], sc[:], mybir.ActivationFunctionType.Exp,
                        scale=scale,
                    )
                    for item in fillers.get(kc, []):
                        item()
                    for hi in range(2):
                        nc.tensor.matmul(
                            avos[hi][:],
                            V_sb[:, kc, 2 * fc + hi, :],
                            pt[:, hi, :],
                            start=(kc == 0),
                            stop=(kc == NT - 1),
                        )
                pass_end(fc, qb, avos)

            # ---- emission schedule (emission order = scheduler priority;
            # actual execution order is dependency-driven, so low-priority
            # h0 items automatically fill the xT-DMA wait gaps) ----
            def KH1(fc, qc, si):
                return lambda: qk_h1(wkT_r, KT_sb, fc, qc, si)

            def QH1(fc, qc, si):
                return lambda: qk_h1(wqT_r, QT_sb, fc, qc, si)

            def QQ(fc, qc, quarter):
                return lambda: qk_q(wqT_r, QT_sb, fc, qc, quarter)

            def VH1(nt):
                return lambda: v_h1(nt)

            def WO(qb, do, tail=False):
                return lambda: wo_item(qb, do, tail)

            # critical chain to the first exp: KT b0 + QT b0 (fc0), whose
            # h1 halves run the moment xT chunk 7 lands.
            warm_up(16)
            qk_h0(wkT_r, 0, 0, 0)
            qk_h1(wkT_r, KT_sb, 0, 0, 0)
            qk_h0(wqT_r, 0, 0, 8)
            qk_h1(wqT_r, QT_sb, 0, 0, 8)
            v_h0(0)
            v_h0(1)
            # bulk h0 halves: only need chunks 0-3 + their weights; they
            # fill the rest of the DMA phase and early-round slack.
            qk_h0(wkT_r, 0, 1, 1)
            qk_h0(wkT_r, 0, 2, 2)
            qk_h0(wkT_r, 0, 3, 3)
            qk_h0(wkT_r, 1, 0, 4)
            qk_h0(wkT_r, 1, 1, 5)
            qk_h0(wkT_r, 1, 2, 6)
            qk_h0(wkT_r, 1, 3, 7)
            qk_h0(wqT_r, 1, 0, 9)
            for nt in range(2, NT):
                v_h0(nt)

            # pass(0,0): V h1 paced 2 chunks ahead of its kc; KT fc0 h1
            # blocks land before their first use (round 4j); KT/QT fc1 b0
            # h1 land before pass(1,0) round 0.
            f00 = {0: [VH1(0), VH1(1), VH1(2)]}
            for kc in range(1, 14):
                f00[kc] = [VH1(kc + 2)]
            f00[1].append(KH1(0, 1, 1))
            f00[5].append(KH1(0, 2, 2))
            f00[9].append(KH1(0, 3, 3))
            f00[11].append(KH1(1, 0, 4))
            f00[13].append(QH1(1, 0, 9))
            do_pass(0, 0, f00)

            # pass(1,0): KT fc1 h1 blocks before their round-4j use; QT
            # qb1 quarter items for both pairs (two blocks straddle the
            # pjps pool on interleaved rounds - exactly 2 held tiles).
            f10 = {
                0: [KH1(1, 1, 5)],
                2: [KH1(1, 2, 6)],
                4: [KH1(1, 3, 7)],
            }
            for i in range(4):
                f10[5 + 2 * i] = [QQ(1, 1, i)]
                f10[6 + 2 * i] = [QQ(0, 1, i)]
            do_pass(1, 0, f10)

            # q-blocks 1..3: wo(qb-1) split 4/4 across the two passes
            # (first use 8 rounds after the normalize chain starts); the
            # next q-block's QT blocks ride as quarter items.
            for qb in range(1, NQB):
                fa = {}
                if qb < NQB - 1:
                    for i in range(4):
                        fa[2 * i] = [QQ(1, qb + 1, i)]
                for i in range(4):
                    fa[8 + 2 * i] = [WO(qb - 1, i)]
                do_pass(0, qb, fa)
                fb = {}
                for i in range(3):
                    fb[2 * i] = [WO(qb - 1, 4 + i)]
                fb[14] = [WO(qb - 1, 7)]
                if qb < NQB - 1:
                    for i in range(4):
                        fb[7 + 2 * i] = [QQ(0, qb + 1, i)]
                do_pass(1, qb, fb)
            # tail: keep the PE warm through the last normalize chain,
            # then the last q-block's Wo with casts on the idle ScalarE.
            warm_up(8)
            for do in range(d // P):
                wo_item(NQB - 1, do, tail=True)
    nc.finalize()
    return nc


def make_in_maps(x, Wq, Wk, Wv, Wo):
    """Shard full inputs into per-core DRAM parameter maps (bf16)."""
    import ml_dtypes

    bf = ml_dtypes.bfloat16
    x = np.asarray(x, dtype=np.float32)
    Wq = np.asarray(Wq, dtype=np.float32)
    Wk = np.asarray(Wk, dtype=np.float32)
    Wv = np.asarray(Wv, dtype=np.float32)
    Wo = np.asarray(Wo, dtype=np.float32)
    xTs = [np.ascontiguousarray(x[b].T).astype(bf) for b in range(B)]
    WqT, WkT, WvT = Wq.T, Wk.T, Wv.T
    in_maps = []
    for c in range(N_CORES):
        b, g = c // (N_CORES // B), c % (N_CORES // B)
        fs = slice(g * F, (g + 1) * F)
        in_maps.append(
            {
                "xT": xTs[b],
                "wqT": np.ascontiguousarray(WqT[:, fs]).astype(bf),
                "wkT": np.ascontiguousarray(WkT[:, fs]).astype(bf),
                "wvT": np.ascontiguousarray(WvT[:, fs]).astype(bf),
                "woT": np.ascontiguousarray(Wo[:, fs].T).astype(bf),
            }
        )
    return in_maps


_NC_CACHE = {}


def run(x, Wq, Wk, Wv, Wo, trace=False, **kw):
    from concourse.bass_utils import run_bass_kernel_spmd

    if "nc" not in _NC_CACHE:
        _NC_CACHE["nc"] = build_nc()
    nc = _NC_CACHE["nc"]
    in_maps = make_in_maps(x, Wq, Wk, Wv, Wo)
    res = run_bass_kernel_spmd(
        nc, in_maps, core_ids=list(range(N_CORES)), trace=trace, **kw
    )
    parts = [
        np.asarray(res.results[i]["out"]).astype(np.float32)
        for i in range(N_CORES)
    ]
    gpb = N_CORES // B
    # per-core partials are transposed [d, n]: sum the group, then untranspose
    full = np.stack(
        [
            sum(parts[b * gpb + 1 : (b + 1) * gpb], parts[b * gpb]).T
            for b in range(B)
        ]
    )
    return np.ascontiguousarray(full, dtype=np.float32), res


def kernel(x, Wq, bq, Wk, bk, Wv, bv, Wo, bo):
    full, _ = run(x, Wq, Wk, Wv, Wo)
    return full


# revision 20
# speedup vs baseline: 1.1858x; 1.0088x over previous
"""Multi-head attention kernel for 8 TRN2 NeuronCores.

Problem: b=2, n=2048, d=1024, heads=16, hd=64.
  q/k/v = x @ W{q,k,v}.T (+ zero bias)
  per head: softmax(q k^T / sqrt(d)) @ v
  out = concat @ Wo.T (+ zero bias)

Sharding (8 cores): data-parallel over batch (2) x tensor-parallel over
heads (16 heads -> 4 groups of 4). Core c handles batch c//4, heads
4*(c%4) .. 4*(c%4)+3 (feature slice of 256 columns). Wo is applied
row-parallel: each core emits a partial output (transposed [d, n],
bf16); the host sums the 4 partials per batch and untransposes.

v3 design (measured HW evolution from the f32r baseline at ~267us and
the v2 rewrite at ~270us):
 - Everything bf16 (same PE rate as f32r, half the DMA + SBUF). Host
   pre-transposes/casts: xT (d,n), w{q,k,v}T (d,256), woT (256,d).
 - Steady state is ACT-paced: per round (head-pair, 512-q block, one
   128-key chunk) the PE does 2 scores matmuls (row-tiled K=64 pair,
   ~390ns) + 2 AV matmuls (~430ns) against one FD=1024 exp (~1114ns).
   The ~365ns/round PE slack is filled by "work items" (projection
   half-blocks, V tiles, Wo chunks) emitted BETWEEN the scores and AV
   matmuls of each round - engine queues are FIFO, so emission order
   controls exactly what the PE does during the exp shadow.
 - Minimal lead-in: only K^T fc0 block0 (streamed behind the xT DMA)
   + Q^T fc0 qb0 + V(0..1) gate the first exp; all other projections
   ride inside passes as work items.
 - Wo for q-block qb is emitted as per-do items inside the next
   q-block's first pass (round >= 4, giving the normalize chain time);
   the last q-block's Wo runs at the tail with its PSUM->SBUF casts on
   ScalarE (idle there) instead of DVE.
 - softmax denominators via the ones-column of V_aug (row hd of avo);
   normalize: copy avo out of PSUM fast, reciprocal in a [128, 4]
   partition-scattered layout via a small SBUF DMA round-trip,
   partition_broadcast on GpSimd, multiply on DVE -> outT bf16.
 - PSUM: scps 2 bufs x [128,2,512]f32 (2 banks each) + avo pair
   (2 banks) + pjps 2 bufs x 1 bank for projection/Wo items = 8 banks.

Biases are structurally zero in this problem spec and are skipped.
"""

import numpy as np

HEADS = 16
D = 1024
N = 2048
B = 2
N_CORES = 8
HPC = HEADS // (N_CORES // B)  # heads per core = 4
HD = D // HEADS                # 64
F = HPC * HD                   # 256 features per core
P = 128


def build_nc(n=N, d=D, hpc=HPC, hd=HD):
    """Build the per-core Bass program (SPMD: same program on all 8 cores)."""
    import concourse.bass as bass
    import concourse.tile as tile
    from concourse import bacc, mybir

    f32 = mybir.dt.float32
    bf16 = mybir.dt.bfloat16
    f = hpc * hd            # per-core feature count (256)
    FC = f // P             # feature chunks / head pairs (2)
    DC = d // P             # contraction chunks over d (8)
    NT = n // P             # key chunks (16)
    QB = 512                # q-block width
    NQB = n // QB           # 4
    scale = 1.0 / float(np.sqrt(np.float32(d)))

    nc = bacc.Bacc("TRN2")

    xT = nc.declare_dram_parameter("xT", [d, n], bf16, isOutput=False)
    wqT = nc.declare_dram_parameter("wqT", [d, f], bf16, isOutput=False)
    wkT = nc.declare_dram_parameter("wkT", [d, f], bf16, isOutput=False)
    wvT = nc.declare_dram_parameter("wvT", [d, f], bf16, isOutput=False)
    woT = nc.declare_dram_parameter("woT", [f, d], bf16, isOutput=False)
    out = nc.declare_dram_parameter("out", [d, n], bf16, isOutput=True)

    xT_c = xT.rearrange("(c p) n -> c p n", p=P)
    wqT_c = wqT.rearrange("(c p) f -> c p f", p=P)
    wkT_c = wkT.rearrange("(c p) f -> c p f", p=P)
    wvT_c = wvT.rearrange("(c p) f -> c p f", p=P)
    woT_c = woT.rearrange("(c p) n -> c p n", p=P)

    with tile.TileContext(nc) as tc:
        with (
            tc.tile_pool(name="qkv", bufs=1) as qkv,
            tc.tile_pool(name="outT", bufs=1) as outp,
            tc.tile_pool(name="pt", bufs=2) as ptp,
            tc.tile_pool(name="norm", bufs=1) as normp,
            tc.tile_pool(name="xw", bufs=1) as xw,
            tc.tile_pool(name="wosb", bufs=4) as wosbp,
            tc.tile_pool(name="scps", bufs=2, space="PSUM") as scps,
            tc.tile_pool(name="avps", bufs=1, space="PSUM") as avps,
            tc.tile_pool(name="pjps", bufs=2, space="PSUM") as pjps,
        ):
            QT_sb = qkv.tile([P, FC, n], bf16)
            KT_sb = qkv.tile([P, FC, n], bf16)
            V_sb = qkv.tile([P, NT, hpc, hd + 1], bf16)
            outT_sb = outp.tile([P, FC, n], bf16)
            woT_sb = outp.tile([P, FC, d], bf16)
            # ones column of V_aug (accumulates softmax denominators in AV)
            ones_c = outp.tile([P, 1], bf16)
            nc.vector.memset(ones_c[:], 1.0)
            nc.vector.tensor_copy(
                V_sb[:, :, :, hd : hd + 1],
                ones_c.to_broadcast([P, NT, hpc, 1]),
            )

            xT_r = xw.tile([P, DC, n], bf16)
            wqT_r = xw.tile([P, DC, f], bf16)
            wkT_r = xw.tile([P, DC, f], bf16)
            wvT_r = xw.tile([P, DC, f], bf16)

            # wk + xT interleaved per chunk (measured faster than issuing
            # the xT stream back-to-back; the queue pipelines them).
            for dc in range(DC):
                nc.sync.dma_start(out=wkT_r[:, dc, :], in_=wkT_c[dc])
                nc.sync.dma_start(out=xT_r[:, dc, :], in_=xT_c[dc])
            for dc in range(DC):
                nc.sync.dma_start(out=wqT_r[:, dc, :], in_=wqT_c[dc])
            for dc in range(DC):
                nc.sync.dma_start(out=wvT_r[:, dc, :], in_=wvT_c[dc])
            for fc in range(FC):
                nc.sync.dma_start(out=woT_sb[:, fc, :], in_=woT_c[fc])

            def warm_up(k):
                # throwaway matmuls on the first wk chunk: keep the PE_HAM
                # activity window busy so real work runs at 2.4 GHz instead
                # of the cold 1.2 GHz default.
                for w in range(k):
                    warm = pjps.tile([P, f], f32, tag="pj", name="warm")
                    nc.tensor.matmul(
                        warm[:],
                        wkT_r[:, 0, 0:P],
                        wkT_r[:, 0, :],
                        start=True,
                        stop=True,
                    )

            # ---- work items ----
            # Lead-in projections are split into two 4-dc halves: h0 (needs
            # xT chunks 0-3 only) accumulates in PSUM, is copied to an SBUF
            # f32 stage and releases its PSUM buffer immediately - so the
            # DMA-paced phase can chew through many h0 items on the 2-buf
            # pjps pool; h1 (chunks 4-7) re-accumulates and DVE-adds the
            # stage into the bf16 destination. Steady-state QT blocks use
            # 2-dc quarter items on one held PSUM tile instead.
            stg = xw.tile([P, 10, QB], f32)     # KT b0-3 x2fc, QT b0 x2fc

            vstg = xw.tile([P, NT, f], f32)
            quarter_state = {}

            def qk_h0(w_sb, fc, qc, si):
                ps = pjps.tile([P, QB], f32, tag="pj")
                sl = slice(qc * QB, (qc + 1) * QB)
                for dc in range(4):
                    nc.tensor.matmul(
                        ps[:],
                        w_sb[:, dc, fc * P : (fc + 1) * P],
                        xT_r[:, dc, sl],
                        start=(dc == 0),
                        stop=(dc == 3),
                    )
                nc.vector.tensor_copy(stg[:, si, :], ps[:])

            def qk_h1(w_sb, dest, fc, qc, si):
                ps = pjps.tile([P, QB], f32, tag="pj")
                sl = slice(qc * QB, (qc + 1) * QB)
                for dc in range(4, DC):
                    nc.tensor.matmul(
                        ps[:],
                        w_sb[:, dc, fc * P : (fc + 1) * P],
                        xT_r[:, dc, sl],
                        start=(dc == 4),
                        stop=(dc == DC - 1),
                    )
                nc.vector.tensor_add(dest[:, fc, sl], stg[:, si, :], ps[:])

            def v_h0(nt):
                ps = pjps.tile([P, QB], f32, tag="pj")
                for dc in range(4):
                    nc.tensor.matmul(
                        ps[:, 0:f],
                        xT_r[:, dc, nt * P : (nt + 1) * P],
                        wvT_r[:, dc, :],
                        start=(dc == 0),
                        stop=(dc == 3),
                    )
                nc.vector.tensor_copy(vstg[:, nt, :], ps[:, 0:f])

            def v_h1(nt):
                ps = pjps.tile([P, QB], f32, tag="pj")
                for dc in range(4, DC):
                    nc.tensor.matmul(
                        ps[:, 0:f],
                        xT_r[:, dc, nt * P : (nt + 1) * P],
                        wvT_r[:, dc, :],
                        start=(dc == 4),
                        stop=(dc == DC - 1),
                    )
                nc.vector.tensor_add(
                    V_sb[:, nt, :, 0:hd],
                    vstg[:, nt, :].rearrange("p (h e) -> p h e", h=hpc),
                    ps[:, 0:f].rearrange("p (h e) -> p h e", h=hpc),
                )

            def qk_q(w_sb, dest, fc, qc, quarter):
                """2-dc quarter of a steady-state projection block; the
                PSUM tile is held across the 4 quarters."""
                key = (id(w_sb), fc, qc)
                sl = slice(qc * QB, (qc + 1) * QB)
                if quarter == 0:
                    ps = pjps.tile([P, QB], f32, tag="pj")
                    quarter_state[key] = ps
                else:
                    ps = quarter_state[key]
                for dc in range(2 * quarter, 2 * quarter + 2):
                    nc.tensor.matmul(
                        ps[:],
                        w_sb[:, dc, fc * P : (fc + 1) * P],
                        xT_r[:, dc, sl],
                        start=(dc == 0),
                        stop=(dc == DC - 1),
                    )
                if quarter == 3:
                    del quarter_state[key]
                    nc.vector.tensor_copy(dest[:, fc, sl], ps[:])

            def wo_item(qb, do, tail=False):
                """One do-chunk of the output projection for q-block qb
                (woT stationary; emits the partial TRANSPOSED [128, QB])."""
                q0 = qb * QB
                ps = pjps.tile([P, QB], f32, tag="pj")
                for fc in range(FC):
                    nc.tensor.matmul(
                        ps[:],
                        woT_sb[:, fc, do * P : (do + 1) * P],
                        outT_sb[:, fc, q0 : q0 + QB],
                        start=(fc == 0),
                        stop=(fc == FC - 1),
                    )
                ob = wosbp.tile([P, QB], bf16, tag="ob")
                if tail:
                    nc.scalar.copy(ob[:], ps[:])
                else:
                    nc.vector.tensor_copy(ob[:], ps[:])
                nc.sync.dma_start(
                    out=out[do * P : (do + 1) * P, q0 : q0 + QB],
                    in_=ob[:],
                )

            def pass_end(fc, qb, avos):
                """Free avo fast, then normalize rows 0..hd-1 by row hd (the
                softmax sums). reciprocal is single-lane-slow on a [1, QB]
                row, so scatter the sums across partitions via a small SBUF
                DMA round-trip first."""
                q0 = qb * QB
                for hi in range(2):
                    po = hi * hd
                    av_sb = normp.tile([hd + 1, QB], f32, tag=f"av_sb{hi}")
                    nc.vector.tensor_copy(av_sb[:], avos[hi][:])
                    rsh = normp.tile([P, QB // P], f32, tag=f"rsh{hi}")
                    nc.sync.dma_start(out=rsh[:], in_=av_sb[hd : hd + 1, :])
                    rsh2 = normp.tile([P, QB // P], f32, tag=f"rsh2{hi}")
                    nc.vector.reciprocal(rsh2[:], rsh[:])
                    recip = normp.tile([1, QB], f32, tag=f"recip{hi}")
                    nc.sync.dma_start(out=recip[:], in_=rsh2[:])
                    bc = normp.tile([hd, QB], f32, tag=f"bc{hi}")
                    nc.gpsimd.partition_broadcast(bc[:], recip[:])
                    nc.vector.tensor_mul(
                        outT_sb[po : po + hd, fc, q0 : q0 + QB],
                        av_sb[0:hd, :],
                        bc[:],
                    )

            def do_pass(fc, qb, fillers):
                """Attention pass for head pair fc (heads 2fc, 2fc+1) on
                q-block qb. fillers[r] = list of work items emitted between
                round r's scores and AV matmuls (they run in the exp
                shadow; the engine queue is FIFO so placement = pacing)."""
                q0 = qb * QB
                avos = [
                    avps.tile([hd + 1, QB], f32, tag=f"avo{i}", name=f"avo{i}")
                    for i in range(2)
                ]
                for kc in range(NT):
                    sc = scps.tile([P, 2, QB], f32, tag="sc")
                    for hi in range(2):
                        p0 = hi * hd
                        nc.tensor.matmul(
                            sc[:, hi, :],
                            KT_sb[p0 : p0 + hd, fc, kc * P : (kc + 1) * P],
                            QT_sb[p0 : p0 + hd, fc, q0 : q0 + QB],
                            start=True,
                            stop=True,
                        )
                    pt = ptp.tile([P, 2, QB], bf16, tag="pt")
                    nc.scalar.activation(
                        pt[:], sc[:], mybir.ActivationFunctionType.Exp,
                        scale=scale,
                    )
                    for item in fillers.get(kc, []):
                        item()
                    for hi in range(2):
                        nc.tensor.matmul(
                            avos[hi][:],
                            V_sb[:, kc, 2 * fc + hi, :],
                            pt[:, hi, :],
                            start=(kc == 0),
                            stop=(kc == NT - 1),
                        )
                pass_end(fc, qb, avos)

            # ---- emission schedule (emission order = scheduler priority;
            # actual execution order is dependency-driven, so low-priority
            # h0 items automatically fill the xT-DMA wait gaps) ----
            def KH1(fc, qc, si):
                return lambda: qk_h1(wkT_r, KT_sb, fc, qc, si)

            def QH1(fc, qc, si):
                return lambda: qk_h1(wqT_r, QT_sb, fc, qc, si)

            def QQ(fc, qc, quarter):
                return lambda: qk_q(wqT_r, QT_sb, fc, qc, quarter)

            def VH1(nt):
                return lambda: v_h1(nt)

            def WO(qb, do, tail=False):
                return lambda: wo_item(qb, do, tail)

            # critical chain to the first exp: KT b0 + QT b0 (fc0), whose
            # h1 halves run the moment xT chunk 7 lands.
            warm_up(16)
            qk_h0(wkT_r, 0, 0, 0)
            qk_h1(wkT_r, KT_sb, 0, 0, 0)
            qk_h0(wqT_r, 0, 0, 8)
            qk_h1(wqT_r, QT_sb, 0, 0, 8)
            v_h0(0)
            v_h0(1)
            # bulk h0 halves: only need chunks 0-3 + their weights; they
            # fill the rest of the DMA phase and early-round slack.
            qk_h0(wkT_r, 0, 1, 1)
            qk_h0(wkT_r, 0, 2, 2)
            qk_h0(wkT_r, 0, 3, 3)
            qk_h0(wkT_r, 1, 0, 4)
            qk_h0(wkT_r, 1, 1, 5)
            qk_h0(wkT_r, 1, 2, 6)
            qk_h0(wkT_r, 1, 3, 7)
            qk_h0(wqT_r, 1, 0, 9)
            for nt in range(2, 8):
                v_h0(nt)

            # pass(0,0): V h1 paced 2 chunks ahead of its kc (the late V
            # h0 halves ride as round items too, so leftover lead work
            # can't outrank round-1+ scores in the PE queue); KT fc0 h1
            # blocks land before their first use (round 4j); KT/QT fc1 b0
            # h1 land before pass(1,0) round 0.
            def VH0(nt):
                return lambda: v_h0(nt)

            f00 = {0: [VH1(0), VH1(1), VH1(2)]}
            for kc in range(1, 14):
                f00[kc] = [VH1(kc + 2)]
            for kc in range(8):
                f00[kc].append(VH0(kc + 8))
            f00[1].append(KH1(0, 1, 1))
            f00[5].append(KH1(0, 2, 2))
            f00[9].append(KH1(0, 3, 3))
            f00[11].append(KH1(1, 0, 4))
            f00[13].append(QH1(1, 0, 9))
            do_pass(0, 0, f00)

            # pass(1,0): KT fc1 h1 blocks before their round-4j use; QT
            # qb1 quarter items for both pairs (two blocks straddle the
            # pjps pool on interleaved rounds - exactly 2 held tiles).
            f10 = {
                0: [KH1(1, 1, 5)],
                2: [KH1(1, 2, 6)],
                4: [KH1(1, 3, 7)],
            }
            for i in range(4):
                f10[5 + 2 * i] = [QQ(1, 1, i)]
                f10[6 + 2 * i] = [QQ(0, 1, i)]
            do_pass(1, 0, f10)

            # q-blocks 1..3: wo(qb-1) split 4/4 across the two passes
            # (first use 8 rounds after the normalize chain starts); the
            # next q-block's QT blocks ride as quarter items.
            for qb in range(1, NQB):
                fa = {}
                if qb < NQB - 1:
                    for i in range(4):
                        fa[2 * i] = [QQ(1, qb + 1, i)]
                for i in range(4):
                    fa[8 + 2 * i] = [WO(qb - 1, i)]
                do_pass(0, qb, fa)
                fb = {}
                for i in range(3):
                    fb[2 * i] = [WO(qb - 1, 4 + i)]
                fb[14] = [WO(qb - 1, 7)]
                if qb < NQB - 1:
                    for i in range(4):
                        fb[7 + 2 * i] = [QQ(0, qb + 1, i)]
                do_pass(1, qb, fb)
            # tail: keep the PE warm through the last normalize chain,
            # then the last q-block's Wo with casts on the idle ScalarE.
            warm_up(8)
            for do in range(d // P):
                wo_item(NQB - 1, do, tail=True)
    nc.finalize()
    return nc


def make_in_maps(x, Wq, Wk, Wv, Wo):
    """Shard full inputs into per-core DRAM parameter maps (bf16)."""
    import ml_dtypes

    bf = ml_dtypes.bfloat16
    x = np.asarray(x, dtype=np.float32)
    Wq = np.asarray(Wq, dtype=np.float32)
    Wk = np.asarray(Wk, dtype=np.float32)
    Wv = np.asarray(Wv, dtype=np.float32)
    Wo = np.asarray(Wo, dtype=np.float32)
    xTs = [np.ascontiguousarray(x[b].T).astype(bf) for b in range(B)]
    WqT, WkT, WvT = Wq.T, Wk.T, Wv.T
    in_maps = []
    for c in range(N_CORES):
        b, g = c // (N_CORES // B), c % (N_CORES // B)
        fs = slice(g * F, (g + 1) * F)
        in_maps.append(
            {
                "xT": xTs[b],
                "wqT": np.ascontiguousarray(WqT[:, fs]).astype(bf),
                "wkT": np.ascontiguousarray(WkT[:, fs]).astype(bf),
                "wvT": np.ascontiguousarray(WvT[:, fs]).astype(bf),
                "woT": np.ascontiguousarray(Wo[:, fs].T).astype(bf),
            }
        )
    return in_maps


_NC_CACHE = {}


def run(x, Wq, Wk, Wv, Wo, trace=False, **kw):
    from concourse.bass_utils import run_bass_kernel_spmd

    if "nc" not in _NC_CACHE:
        _NC_CACHE["nc"] = build_nc()
    nc = _NC_CACHE["nc"]
    in_maps = make_in_maps(x, Wq, Wk, Wv, Wo)
    res = run_bass_kernel_spmd(
        nc, in_maps, core_ids=list(range(N_CORES)), trace=trace, **kw
    )
    parts = [
        np.asarray(res.results[i]["out"]).astype(np.float32)
        for i in range(N_CORES)
    ]
    gpb = N_CORES // B
    # per-core partials are transposed [d, n]: sum the group, then untranspose
    full = np.stack(
        [
            sum(parts[b * gpb + 1 : (b + 1) * gpb], parts[b * gpb]).T
            for b in range(B)
        ]
    )
    return np.ascontiguousarray(full, dtype=np.float32), res


def kernel(x, Wq, bq, Wk, bk, Wv, bv, Wo, bo):
    full, _ = run(x, Wq, Wk, Wv, Wo)
    return full
